# revision 9
# baseline (speedup 1.0000x reference)
"""Trainium2 Bass kernel for nn_CustomLoss_62921270887106.

Loss = BCE(class_pred, class_gt) (mean, torch log-clamp at -100)
     + mean_b( 0.5 * sum_jc[ (class_pred>=0.5) * (reg_pred-reg_gt)^2 ] / (1 + sum_j class_gt) )

Strategy: pure data parallel over the batch dim on 8 NeuronCores.
Each core reduces its 125000-sample shard to per-partition partial sums
[128, 2] (col0: sum of BCE log-terms, col1: sum of 0.5*sq/nj); the host
sums the 8x128 partials in float64 and combines.

Key per-core pipeline (sample-major layout, K=61 samples per partition
per tile, 16 main tiles of 7808 samples + one 72-sample tail tile):
  u    = (p - 1) + g                      [DVE scalar_tensor_tensor]
  t    = |u|  (== p if g==1 else 1-p)     [ACT Abs]
  L    = ln(t + 2e-38), accum -> bce col  [ACT Ln with accum_out]
  diff = rp - rg                          [DVE tensor_sub]
  d2   = diff^2                           [ACT Square]
  md   = (p >= 0.5) * d2                  [DVE scalar_tensor_tensor, is_ge+mult]
  sq   = reduce_X md  [128,61,34]->[128,61]
  njs  = reduce_X g   [128,61,17]->[128,61]
Epilogue: 1/nj via exp(-ln(nj)) on ACT, 0.5*sq*rnj via tensor_tensor_reduce.
"""

import sys

for _p in ("/opt/trn_rl_repo",):
    if _p not in sys.path:
        sys.path.insert(0, _p)

import numpy as np

import concourse.bass as bass
import concourse.tile as tile
from concourse import bacc, mybir
from concourse.bass_utils import run_bass_kernel_spmd

F32 = mybir.dt.float32
AF = mybir.ActivationFunctionType
ALU = mybir.AluOpType
AX = mybir.AxisListType

B = 1_000_000
J = 17
C = 3
N_CORES = 8
N_LOC = B // N_CORES            # 125000 samples per core
P = 128
K = 61                          # samples per partition per main tile
M = J * C                       # 51 floats per sample

_PROGRAM_CACHE = {}


def _build_program(n_loc=N_LOC):
    TILE_SAMPLES = P * K             # 7808
    NT_MAIN = n_loc // TILE_SAMPLES
    MAIN = NT_MAIN * TILE_SAMPLES
    TAIL = n_loc - MAIN
    NCOLS = NT_MAIN * K + 1          # sq/nj buffer columns
    N_LOC_ = n_loc
    nc = bacc.Bacc("TRN2", target_bir_lowering=False, debug=False,
                   num_devices=N_CORES)

    o_dram = nc.dram_tensor("output", [N_LOC_, J, C], F32, kind="ExternalInput").ap()
    t_dram = nc.dram_tensor("target", [N_LOC_, J, C], F32, kind="ExternalInput").ap()
    partials = nc.dram_tensor("partials", [P, 2], F32, kind="ExternalOutput").ap()

    o_flat = o_dram.rearrange("b j c -> b (j c)")
    t_flat = t_dram.rearrange("b j c -> b (j c)")
    o_main = o_flat[0:MAIN, :].rearrange("(n p k) m -> n p (k m)", p=P, k=K)
    t_main = t_flat[0:MAIN, :].rearrange("(n p k) m -> n p (k m)", p=P, k=K)
    o_tail = o_flat[MAIN:N_LOC_, :]   # [72, 51]
    t_tail = t_flat[MAIN:N_LOC_, :]

    with tile.TileContext(nc) as tc:
        with (
            tc.tile_pool(name="inp", bufs=3) as inp,
            tc.tile_pool(name="work", bufs=2) as work,
            tc.tile_pool(name="persist", bufs=1) as persist,
        ):
            sqbuf = persist.tile([P, NCOLS], F32)
            njbuf = persist.tile([P, NCOLS], F32)
            bcecols = persist.tile([P, NT_MAIN + 1], F32)
            outtile = persist.tile([P, 2], F32)
            bias_eps = persist.tile([P, 1], F32)

            nc.gpsimd.memset(sqbuf[:], 0.0)
            nc.gpsimd.memset(njbuf[:], 0.0)
            nc.gpsimd.memset(bcecols[:], 0.0)
            nc.gpsimd.memset(bias_eps[:], 2e-38)

            def do_tile(o_src, t_src, rows, k, t_idx, sq_dst, nj_dst, bce_dst):
                # o_src/t_src: DRAM APs [rows, k*M]
                to = inp.tile([P, k * M], F32, tag="to")
                tt = inp.tile([P, k * M], F32, tag="tt")
                nc.sync.dma_start(out=to[:rows, :], in_=o_src)
                nc.sync.dma_start(out=tt[:rows, :], in_=t_src)

                o4 = to[:rows, :].rearrange("p (k j c) -> p k j c", k=k, j=J, c=C)
                t4 = tt[:rows, :].rearrange("p (k j c) -> p k j c", k=k, j=J, c=C)
                p_flat = o4[:, :, :, 2].rearrange("p k j -> p (k j)")   # [rows, k*J]
                g_flat = t4[:, :, :, 2].rearrange("p k j -> p (k j)")
                rp = o4[:, :, :, 0:2]                                   # [rows, k, J, 2]
                rg = t4[:, :, :, 0:2]
                p_b = o4[:, :, :, 2:3].broadcast_to([rows, k, J, 2])

                # BCE: u = (p - 1) + g ; t = |u| ; L = ln(t + 2e-38) with accum
                u = work.tile([P, k * J], F32, tag="u")
                nc.vector.scalar_tensor_tensor(
                    out=u[:rows, :], in0=p_flat, scalar=1.0, in1=g_flat,
                    op0=ALU.subtract, op1=ALU.add,
                )
                tabs = work.tile([P, k * J], F32, tag="tabs")
                nc.scalar.activation(tabs[:rows, :], u[:rows, :], AF.Abs)
                lnt = work.tile([P, k * J], F32, tag="lnt")
                nc.scalar.activation(lnt[:rows, :], tabs[:rows, :], AF.Ln,
                                     bias=bias_eps[:rows, 0:1], accum_out=bce_dst)

                # masked squared diff
                diff = work.tile([P, k, J, 2], F32, tag="diff")
                nc.vector.tensor_sub(diff[:rows], rp, rg)
                d2 = work.tile([P, k, J, 2], F32, tag="d2")
                nc.scalar.activation(d2[:rows], diff[:rows], AF.Square)
                md = work.tile([P, k, J, 2], F32, tag="md")
                nc.vector.scalar_tensor_tensor(
                    out=md[:rows], in0=p_b, scalar=0.5, in1=d2[:rows],
                    op0=ALU.is_ge, op1=ALU.mult,
                )

                # per-sample reductions
                md3 = md[:rows].rearrange("p k j c -> p k (j c)")
                nc.vector.tensor_reduce(sq_dst, md3, axis=AX.X, op=ALU.add)
                g3 = t4[:, :, :, 2]                                     # [rows, k, J]
                nc.vector.tensor_reduce(nj_dst, g3, axis=AX.X, op=ALU.add)

            for t in range(NT_MAIN):
                do_tile(
                    o_main[t], t_main[t], P, K, t,
                    sq_dst=sqbuf[:, t * K:(t + 1) * K],
                    nj_dst=njbuf[:, t * K:(t + 1) * K],
                    bce_dst=bcecols[:, t:t + 1],
                )
            # tail: 72 samples, one sample per partition (k=1)
            if TAIL > 0:
                do_tile(
                    o_tail, t_tail, TAIL, 1, NT_MAIN,
                    sq_dst=sqbuf[:TAIL, NCOLS - 1:NCOLS],
                    nj_dst=njbuf[:TAIL, NCOLS - 1:NCOLS],
                    bce_dst=bcecols[:TAIL, NT_MAIN:NT_MAIN + 1],
                )

            # epilogue: wsum = sum_cols sq / (2 * (1 + nj))
            njp = persist.tile([P, NCOLS], F32)
            nc.vector.tensor_scalar_add(njp[:], njbuf[:], 1.0)
            lnn = persist.tile([P, NCOLS], F32)
            nc.scalar.activation(lnn[:], njp[:], AF.Ln, scale=2.0)  # ln(2*nj)
            rnj = persist.tile([P, NCOLS], F32)
            nc.scalar.activation(rnj[:], lnn[:], AF.Exp, scale=-1.0)  # 1/(2*nj)
            wd = persist.tile([P, NCOLS], F32)
            nc.vector.tensor_mul(wd[:], sqbuf[:], rnj[:])
            nc.vector.tensor_reduce(outtile[:, 1:2], wd[:], axis=AX.X,
                                    op=ALU.add)
            nc.vector.tensor_reduce(outtile[:, 0:1], bcecols[:], axis=AX.X,
                                    op=ALU.add)
            nc.sync.dma_start(out=partials, in_=outtile[:])

    nc.compile()
    return nc


def _get_program(n_loc=N_LOC):
    if n_loc not in _PROGRAM_CACHE:
        _PROGRAM_CACHE[n_loc] = _build_program(n_loc)
    return _PROGRAM_CACHE[n_loc]


def _run_shards(output, target, trace=False, **kw):
    nc = _get_program()
    o = np.ascontiguousarray(np.asarray(output, dtype=np.float32))
    t = np.ascontiguousarray(np.asarray(target, dtype=np.float32))
    in_maps = []
    for i in range(N_CORES):
        sl = slice(i * N_LOC, (i + 1) * N_LOC)
        in_maps.append({"output": o[sl], "target": t[sl]})
    return run_bass_kernel_spmd(nc, in_maps, list(range(N_CORES)),
                                trace=trace, **kw)


def _combine(results):
    bce_sum = 0.0
    wsq_sum = 0.0
    for r in results:
        p = np.asarray(r["partials"], dtype=np.float64)
        bce_sum += p[:, 0].sum()
        wsq_sum += p[:, 1].sum()
    loss = -bce_sum / (B * J) + wsq_sum / B
    return np.float32(loss)


def kernel(output, target):
    res = _run_shards(output, target, trace=False)
    return _combine(res.results)
